# revision 37
# baseline (speedup 1.0000x reference)
"""Trainium2 Bass kernel for nn_AttentionBlock (GroupNorm + 1x1-conv QKV +
full self-attention over N=HW=4096 + output projection + residual).

Distribution: data-parallel over batch B=8, one batch element per NeuronCore.

v3 architecture (fp8 attention probabilities + DoubleRow matmuls):
  - S = K^T Q stays bf16: it is PSUM-write bound at 1 col/cycle (131K
    cycles/core) in every dtype/perf-mode (measured), so fp8 buys nothing.
  - exp(S - 1.5) is split per j-tile across two engines: ACT does columns
    [0:EA) via the Exp table with fp8e4 output (the -1.5 shift keeps
    exp below the TRN e4m3 +/-240->Inf ceiling and cancels in softmax);
    DVE does [EA:1024) via a Schraudolph bit-trick: one tensor_scalar
    computes rne(x*(8/ln2) + b) into a uint8 view of the fp8 tile
    (f32->uint8 convert saturates negatives to 0 == fp8 +0).
  - O accumulates with perf_mode=DoubleRow: lhsT = "V"^T key-pairs
    [128,2,128] fp8, rhs = P pairs [128,2,512] fp8.  DR runs at the same
    1 col/cycle but with 256-deep contraction -> 2x MACs (measured 216ns
    per [128,512] instruction, same as bf16).
  - The projection is folded into the attention matmul: the stationary
    is U^T with U = w_proj @ w_v (host-precomputed), so the O accumulator
    directly holds proj(attention) * d.
  - The softmax denominator comes from a second DoubleRow matmul with an
    all-ones stationary: out [128,512] = column sums of P replicated
    across all partitions (free broadcast), PSUM-accumulated over key
    pairs.  These d-matmuls are deferred behind a >=4-pair backlog so
    they never wait on a just-finished exp.
  - Per-block output chain: d -> [1,1024] row copy -> DMA-scatter to
    [128,8] -> DVE reciprocal (direct reciprocal is ~6.6 cyc/elem; the
    scatter makes it 8 elems/lane) -> DMA back -> PE ones-row broadcast
    matmul into the vacated d banks -> out = (usb * bps + beff) + x via
    one tensor_tensor + one scalar_tensor_tensor on DVE.
  - K bias is dropped entirely (a per-query constant in S cancels in
    softmax).  K/Q/"V" emission: h=0 in the preamble, the rest
    interleaved into block 0's j-loop through the d half-bank slots so
    the engines' copy backlog does not delay exp(0).

PSUM: s0,s1 (S double-buffer) + o (O accum) + d (denominator accum) =
8 banks.  The broadcast matmuls reuse the d banks in the inter-block
window between the reciprocal read and the next block's deferred
d-matmuls.

Engine facts this schedule is built around (measured on hw):
  - every matmul flavor (bf16 / fp8 / fp8+DoubleRow / DoublePixel)
    streams moving columns at 1 col/cycle @2.4GHz; DR doubles the
    contraction per column; LDWEIGHTS mostly hides in the PE reorder
    window.
  - gpsimd (Pool) tensor ops are ~15 ns/col software Q7 ops: useless
    for bulk elementwise; it cannot touch PSUM either.
  - ACT ~0.83 ns/col + ~185ns fixed; DVE ~1.04 ns/col + ~125ns fixed.
"""

import numpy as np

B, C, H, W = 8, 128, 64, 64
HW = H * W                      # 4096
GROUPS = 8
GSIZE = C // GROUPS             # 16
EPS = 1e-5
NJ = HW // 128                  # 32 j-tiles
NJP = NJ // 2                   # 16 key pairs
QW = 1024                       # queries per block
NQT = HW // QW                  # 4 blocks
SCALE = float(C) ** -0.5
CSH = 1.5                       # exp shift, cancels in softmax
A8 = 11.541560327111707         # 8/ln2
B8 = 55.55 - CSH * A8           # schraudolph bias, rne-calibrated

_CACHE = {}


def _patch_ldw_opt():
    import concourse.bass_utils as _bu
    if getattr(_bu, "_ldw_opt_patched", False):
        return
    _orig = _bu.run_command

    def _patched(cmd, *a, **kw):
        if isinstance(cmd, list):
            pass
        return _orig(cmd, *a, **kw)

    _bu.run_command = _patched
    _bu._ldw_opt_patched = True


def _build():
    from contextlib import ExitStack

    import concourse.bacc as bacc
    import concourse.tile as tile
    from concourse import mybir

    f32 = mybir.dt.float32
    bf16 = mybir.dt.bfloat16
    fp8 = mybir.dt.float8e4
    u8 = mybir.dt.uint8
    AF = mybir.ActivationFunctionType
    PM = mybir.MatmulPerfMode
    ALU = mybir.AluOpType

    _patch_ldw_opt()
    nc = bacc.Bacc("TRN2", target_bir_lowering=False, debug=False)

    x_in = nc.dram_tensor("x", [C, HW], f32, kind="ExternalInput")
    gamma_in = nc.dram_tensor("gamma", [C, 1], f32, kind="ExternalInput")
    beta_in = nc.dram_tensor("beta", [C, 1], f32, kind="ExternalInput")
    bq_in = nc.dram_tensor("bq", [C, 1], f32, kind="ExternalInput")
    beff_in = nc.dram_tensor("beff", [C, 1], f32, kind="ExternalInput")
    wq_in = nc.dram_tensor("wqT", [C, C], f32, kind="ExternalInput")
    wk_in = nc.dram_tensor("wkT", [C, C], f32, kind="ExternalInput")
    wu_in = nc.dram_tensor("wuT", [C, C], f32, kind="ExternalInput")
    ig_in = nc.dram_tensor("ig", [C, GROUPS], f32, kind="ExternalInput")
    igt_in = nc.dram_tensor("igt", [GROUPS, C], f32, kind="ExternalInput")
    out_dram = nc.dram_tensor("out", [C, HW], f32, kind="ExternalOutput")

    with tile.TileContext(nc) as tc, ExitStack() as ctx:
        const = ctx.enter_context(tc.tile_pool(name="const", bufs=1))
        big = ctx.enter_context(tc.tile_pool(name="big", bufs=1))
        stats = ctx.enter_context(tc.tile_pool(name="stats", bufs=1))
        ptpool = ctx.enter_context(tc.tile_pool(name="pt", bufs=12))
        osbp = ctx.enter_context(tc.tile_pool(name="osb", bufs=2))
        rcpp = ctx.enter_context(tc.tile_pool(name="rcp", bufs=2))
        onrmp = ctx.enter_context(tc.tile_pool(name="onrm", bufs=3))
        ostp = ctx.enter_context(tc.tile_pool(name="ostg", bufs=4))
        ps = ctx.enter_context(tc.tile_pool(name="ps", bufs=1, space="PSUM"))

        # ---------------- x load (4 DMA queues), consts ----------------
        x_sb = big.tile([C, HW], f32, tag="x")
        xq = [nc.sync, nc.scalar, nc.gpsimd]
        NXC = 8
        XCW = HW // NXC
        for ch in range(NXC):
            sl = slice(ch * XCW, (ch + 1) * XCW)
            xq[ch % 3].dma_start(x_sb[:, sl], x_in[:, sl])

        def cload(t_in, shape, tag):
            t = const.tile(shape, f32, tag=tag)
            nc.scalar.dma_start(t[:], t_in[:])
            return t

        gamma = cload(gamma_in, [C, 1], "c_gamma")
        beta = cload(beta_in, [C, 1], "c_beta")
        bq = cload(bq_in, [C, 1], "c_bq")
        beff = cload(beff_in, [C, 1], "c_beff")
        ig = cload(ig_in, [C, GROUPS], "c_ig")
        igt = cload(igt_in, [GROUPS, C], "c_igt")
        wq_f = cload(wq_in, [C, C], "c_wq_f")
        wk_f = cload(wk_in, [C, C], "c_wk_f")
        wu_f = cload(wu_in, [C, C], "c_wu_f")

        with nc.allow_low_precision(reason="fp8 ones/bias consts"):
            ones8 = const.tile([128, 2, C], fp8)
            nc.gpsimd.memset(ones8[:], 1.0)
            ones_r = const.tile([1, C], bf16)
            nc.gpsimd.memset(ones_r[:], 1.0)

        negc = const.tile([C, 1], f32)
        nc.gpsimd.memset(negc[:], -CSH)
        eps_t = const.tile([GROUPS, 1], f32)
        nc.vector.memset(eps_t[:], EPS)
        magic_t = const.tile([GROUPS, 1], mybir.dt.uint32)
        nc.vector.memset(magic_t[:], 0x5F3759DF)
        c15_t = const.tile([GROUPS, 1], f32)
        nc.vector.memset(c15_t[:], 1.5)

        with nc.allow_low_precision(reason="bf16 weights: rel tol is 2e-2"):
            wk_b = const.tile([C, C], bf16)
            nc.vector.tensor_copy(wk_b[:], wk_f[:])
            wq_b = const.tile([C, C], bf16)
            nc.vector.tensor_copy(wq_b[:], wq_f[:])
            wu_b = const.tile([C, C], bf16)
            nc.vector.tensor_copy(wu_b[:], wu_f[:])

        # ---------------- groupnorm stats via bn_stats ----------------
        bnst = stats.tile([C, 8, 6], f32)
        for ch in range(8):
            sl = slice(ch * 512, (ch + 1) * 512)
            nc.vector.bn_stats(bnst[:, ch, :], x_sb[:, sl])
        mv = stats.tile([C, 2], f32)
        nc.vector.bn_aggr(mv[:], bnst[:])
        warm = stats.tile([GROUPS, 1], f32)
        nc.scalar.activation(warm[:], eps_t[:], AF.Exp)

        msq = stats.tile([C, 2], f32)
        nc.vector.tensor_copy(msq[:, 0:1], mv[:, 0:1])
        nc.vector.tensor_mul(msq[:, 1:2], mv[:, 0:1], mv[:, 0:1])
        nc.vector.tensor_add(msq[:, 1:2], msq[:, 1:2], mv[:, 1:2])

        # persistent PSUM tiles: the whole loop's working set (8 banks)
        s_ps = [None, None]

        def fetch_s(k):
            s_ps[k] = ps.tile([C, QW], f32, tag=f"s{k}", name=f"s{k}")
            return s_ps[k]

        fetch_s(0)
        fetch_s(1)

        gs_ps = s_ps[0][0:GROUPS, 0:2]
        nc.tensor.matmul(gs_ps, ig[:], msq[:], start=True, stop=True)
        gmr = stats.tile([GROUPS, 2], f32)
        nc.vector.tensor_copy(gmr[:, 0:1], gs_ps[:, 0:1])
        gmsq = stats.tile([GROUPS, 1], f32)
        nc.vector.tensor_mul(gmsq[:], gmr[:, 0:1], gmr[:, 0:1])
        gve = stats.tile([GROUPS, 1], f32)
        nc.vector.tensor_sub(gve[:], gs_ps[:, 1:2], gmsq[:])
        nc.vector.tensor_scalar(
            gve[:], gve[:], eps_t[:], None, ALU.add
        )
        # rstd = rsqrt(var+eps): quake guess + 1 Newton step (avoids an
        # ACT Sqrt table load that would evict/delay the Exp table)
        u32 = mybir.dt.uint32
        gu = stats.tile([GROUPS, 1], u32)
        nc.vector.tensor_scalar(
            gu[:], gve[:].bitcast(u32), 1, None,
            ALU.logical_shift_right,
        )
        nc.vector.tensor_sub(gu[:], magic_t[:], gu[:])
        gy = stats.tile([GROUPS, 1], f32)
        nc.vector.tensor_copy(gy[:], gu[:].bitcast(f32))
        gh = stats.tile([GROUPS, 1], f32)
        nc.vector.tensor_scalar_mul(gh[:], gve[:], 0.5)
        gt = stats.tile([GROUPS, 1], f32)
        nc.vector.tensor_mul(gt[:], gy[:], gy[:])
        nc.vector.tensor_mul(gt[:], gt[:], gh[:])
        nc.vector.tensor_sub(gt[:], c15_t[:], gt[:])
        nc.vector.tensor_mul(gmr[:, 1:2], gy[:], gt[:])

        bc_ps = s_ps[1][0:C, 0:2]
        nc.tensor.matmul(bc_ps, igt[:], gmr[:], start=True, stop=True)
        a_c = stats.tile([C, 1], f32)
        b_c = stats.tile([C, 1], f32)
        tmc = stats.tile([C, 1], f32)
        nc.vector.tensor_scalar_mul(a_c[:], gamma[:], bc_ps[:, 1:2])
        nc.vector.tensor_scalar_mul(tmc[:], a_c[:], bc_ps[:, 0:1])
        nc.vector.tensor_sub(b_c[:], beta[:], tmc[:])


        # ---------------- hn, and all of K/Q/V (preamble) ----------------
        hn = big.tile([C, HW], bf16, tag="hn")
        q_r = big.tile([C, HW], bf16, tag="q")
        k_r = big.tile([C, HW], bf16, tag="k")
        vtp = big.tile([128, HW], fp8, tag="vtp")  # V^T, key-major

        lp = nc.allow_low_precision(reason="fp8/bf16 data path: tol 2e-2")
        lp.__enter__()

        # hn halves: ACT does odd (affine via scale+bias), DVE even
        for h in range(8):
            sl = slice(h * 512, (h + 1) * 512)
            if h % 2 == 1:
                nc.scalar.activation(
                    hn[:, sl], x_sb[:, sl], AF.Identity,
                    bias=b_c[:], scale=a_c[:],
                )
            else:
                nc.vector.tensor_scalar(
                    hn[:, sl], x_sb[:, sl], a_c[:], b_c[:],
                    ALU.mult, ALU.add,
                )

        # 24 psum stagings rotate through 8 half-bank slots
        o_ps_pre = ps.tile([C, QW], f32, tag="o")
        d_ps_pre = ps.tile([C, QW], f32, tag="d")
        slots = []
        for t in (s_ps[0], s_ps[1], o_ps_pre, d_ps_pre):
            slots.append(t[:, 0:512])
            slots.append(t[:, 512:1024])
        sidx = 0

        def stage():
            nonlocal sidx
            st = slots[sidx % 8]
            sidx += 1
            return st

        # K: plain copy (k-bias is a per-query constant in S -> cancels)
        # Q: bias bq (pre-scaled);  V^T: direct per-tile matmuls, fp8 copy.
        # Only the h=0 pieces (needed at j0) run in the preamble; the other
        # 20 halves are interleaved into block 0's j-loop through the d
        # half-bank staging slots, so the engines' copy backlog does not
        # delay exp(0).
        def emit_k(h, stg):
            sl = slice(h * 512, (h + 1) * 512)
            nc.tensor.matmul(stg, wk_b[:], hn[:, sl], start=True, stop=True)
            if h % 2 == 0:
                nc.scalar.activation(k_r[:, sl], stg, AF.Copy)
            else:
                nc.vector.tensor_copy(k_r[:, sl], stg)

        def emit_q(h, stg):
            sl = slice(h * 512, (h + 1) * 512)
            nc.tensor.matmul(stg, wq_b[:], hn[:, sl], start=True, stop=True)
            if h % 2 == 0:
                nc.scalar.activation(
                    q_r[:, sl], stg, AF.Identity, bias=bq[:]
                )
            else:
                nc.vector.tensor_scalar(
                    q_r[:, sl], stg, bq[:], None, ALU.add
                )

        def emit_v(h, stg):
            sl = slice(h * 512, (h + 1) * 512)
            for t in range(4):
                nt = h * 4 + t
                nc.tensor.matmul(
                    stg[:, t * 128:(t + 1) * 128],
                    hn[:, nt * 128:(nt + 1) * 128], wu_b[:],
                    start=True, stop=True,
                )
            if h % 2 == 1:
                nc.scalar.activation(vtp[:, sl], stg, AF.Copy)
            else:
                nc.vector.tensor_copy(vtp[:, sl], stg)

        emit_k(0, stage())
        emit_q(0, stage())
        emit_q(1, stage())
        emit_v(0, stage())

        # deferred aux work for block 0: (kind, h) pairs in deadline order
        aux_sched = {}
        _items = []
        for h in range(1, 8):
            _items.append(("k", h))
            _items.append(("v", h))
        for h in range(2, 8):
            _items.append(("q", h))
        for j in range(20):
            aux_sched[j] = [_items[j]]

        def vtp_pair(jp):
            return vtp[:, jp * 256:(jp + 1) * 256].rearrange(
                "p (two f) -> p two f", two=2)

        # ---------------- main attention loop ----------------
        # ACT exp column share per j; reduced on js where the block chain
        # puts extra work on ACT/DVE.
        chain = {}
        pend_d = []     # deferred denominator DR-matmul groups
        d_cnt = [0]     # groups issued for current block's d accumulation
        cur_d = [None]  # current block's d accumulator (lazy fetch: the
                        # banks are vacated by the prev block's proj first)
        cur_qt = [0]

        def flush_d(maxn, minkeep=4):
            n = 0
            while len(pend_d) > minkeep and n < maxn:
                if cur_d[0] is None:
                    cur_d[0] = d_ps_pre if cur_qt[0] == 0 else \
                        ps.tile([C, QW], f32, tag="d", name="d_acc")
                pt, first = pend_d.pop(0)
                for cch in range(2):
                    csl = slice(cch * 512, (cch + 1) * 512)
                    nc.tensor.matmul(
                        cur_d[0][:, csl], ones8[:], pt[:, :, csl],
                        start=first, stop=(d_cnt[0] == NJP - 1),
                        perf_mode=PM.DoubleRow,
                    )
                d_cnt[0] += 1
                n += 1

        cur_o = [None]
        cur_o_qt = [0]

        def get_o():
            if cur_o[0] is None:
                cur_o[0] = o_ps_pre if cur_o_qt[0] == 0 else \
                    ps.tile([C, QW], f32, tag="o", name="o_acc")
            return cur_o[0]

        def emit_O(jp, pt):
            o_ps = get_o()
            for cch in range(2):
                csl = slice(cch * 512, (cch + 1) * 512)
                nc.tensor.matmul(
                    o_ps[:, csl], vtp_pair(jp), pt[:, :, csl],
                    start=(jp == 0), stop=(jp == NJP - 1),
                    perf_mode=PM.DoubleRow,
                )

        def chain_ops(qt, j):
            # output chain for block qt-1, emitted at fixed js of block qt.
            # The O accumulator already holds the PROJECTED output times d
            # (U = w_proj @ w_v folded on host), so after the reciprocal
            # broadcast the output is just (usb * bps) + xb on DVE.
            pq = qt - 1
            dpv = chain["d_prev"]
            if j in (0, 1):
                # deferred usb copies: queued after this block's first exps
                # but emitted before the first emit_O (j5) so the new o-tile
                # epoch (lazy fetch) orders its writes after these reads
                csl = slice(j * 512, (j + 1) * 512)
                if j == 0:
                    nc.scalar.activation(
                        chain["usb_prev"][:, csl], chain["o_prev"][:, csl],
                        AF.Copy)
                else:
                    nc.vector.tensor_copy(
                        chain["usb_prev"][:, csl], chain["o_prev"][:, csl])
            elif j in (2, 3):
                cch = j - 2
                csl = slice(cch * 512, (cch + 1) * 512)
                if cch == 0:
                    drow = rcpp.tile([1, QW], f32)
                    chain["drow"] = drow
                    rs = rcpp.tile([128, 8], f32, tag="rs", name="rs")
                    chain["rs"] = rs
                    nc.scalar.activation(drow[0:1, csl], dpv[0:1, csl],
                                         AF.Copy)
                else:
                    drow = chain["drow"]
                    nc.vector.tensor_copy(drow[0:1, csl], dpv[0:1, csl])
                nc.sync.dma_start(
                    chain["rs"][:, cch * 4:(cch + 1) * 4], drow[0:1, csl])
            elif j in (4, 5):
                cch = j - 4
                if cch == 0:
                    rc = rcpp.tile([128, 8], bf16, tag="rc", name="rc")
                    chain["rc"] = rc
                    rrow = rcpp.tile([1, QW], bf16, tag="rrow", name="rrow")
                    chain["rrow"] = rrow
                nc.vector.reciprocal(
                    chain["rc"][:, cch * 4:(cch + 1) * 4],
                    chain["rs"][:, cch * 4:(cch + 1) * 4])
                nc.sync.dma_start(
                    chain["rrow"][0:1, cch * 512:(cch + 1) * 512],
                    chain["rc"][:, cch * 4:(cch + 1) * 4])
            elif j in (6, 7):
                cch = j - 6
                csl = slice(cch * 512, (cch + 1) * 512)
                nc.tensor.matmul(
                    dpv[:, csl], ones_r[:], chain["rrow"][0:1, csl],
                    start=True, stop=True,
                )
            elif j in (8, 9):
                cch = j - 8
                csl = slice(cch * 512, (cch + 1) * 512)
                tno = onrmp.tile([C, 512], bf16, name="tno")
                nc.vector.tensor_tensor(
                    tno[:], chain["usb_prev"][:, csl], dpv[:, csl],
                    ALU.mult,
                )
                chain[f"tno{cch}"] = tno
            elif j in (10, 11):
                cch = j - 10
                csl = slice(cch * 512, (cch + 1) * 512)
                osl = slice(pq * QW + cch * 512, pq * QW + (cch + 1) * 512)
                ost = ostp.tile([C, 512], f32)
                nc.vector.scalar_tensor_tensor(
                    ost[:], chain[f"tno{cch}"][:], beff[:], x_sb[:, osl],
                    ALU.add, ALU.add,
                )
                if qt > NQT - 1:
                    # final tail: split the store across two queues
                    h0 = slice(pq * QW + cch * 512, pq * QW + cch * 512 + 256)
                    nc.sync.dma_start(out_dram[:, h0], ost[:, 0:256])
                    h1 = slice(pq * QW + cch * 512 + 256,
                               pq * QW + (cch + 1) * 512)
                    nc.gpsimd.dma_start(out_dram[:, h1], ost[:, 256:512])
                else:
                    nc.sync.dma_start(out_dram[:, osl], ost[:])

        def ea_of(qt, j):
            if qt > 0:
                if j in (0, 2):
                    return 320      # ACT does usb/drow copies
                if j == 1:
                    return 768      # DVE usb copy
                if j in (3, 8, 9, 10, 11):
                    return 768      # DVE chain ops
                if j in (4, 5):
                    return 640
            return 512

        for qt in range(NQT):
            usb = osbp.tile([C, QW], bf16)
            cur_o[0] = None
            cur_o_qt[0] = qt
            pts = {}
            d_cnt[0] = 0
            cur_d[0] = None
            cur_qt[0] = qt
            for j in range(NJ):
                jp = j // 2
                qoff = qt * QW

                # --- S pair
                sp = ps.tile([C, QW], f32, tag=f"s{j % 2}", name=f"sp{j % 2}")
                nc.tensor.matmul(
                    sp[:, 0:512], k_r[:, j * 128:(j + 1) * 128],
                    q_r[:, qoff:qoff + 512], start=True, stop=True,
                )
                nc.tensor.matmul(
                    sp[:, 512:1024], k_r[:, j * 128:(j + 1) * 128],
                    q_r[:, qoff + 512:qoff + 1024], start=True, stop=True,
                )
                # --- pipelined O(jp-2) after the odd-j S pair (two-deep:
                # the engines' exp stream lags the PE by about one pair)
                if j % 2 == 1 and jp >= 2:
                    emit_O(jp - 2, pts[jp - 2])
                # --- block-0 staging backlog through the d half-slots
                if qt == 0 and j in aux_sched:
                    for kind, h in aux_sched[j]:
                        stg = d_ps_pre[:, (h % 2) * 512:(h % 2 + 1) * 512]
                        dict(k=emit_k, q=emit_q, v=emit_v)[kind](h, stg)
                # --- deferred denominator matmuls (after the staging /
                # chain projection work vacates the d banks)
                if j >= (21 if qt == 0 else 14):
                    flush_d(2, minkeep=(2 if qt == NQT - 1 and j >= 24 else 4))

                # --- exp split ACT/DVE
                if j % 2 == 0:
                    pt = ptpool.tile([128, 2, QW], fp8)
                    pts[jp] = pt
                else:
                    pt = pts[jp]
                i = j % 2
                ea = ea_of(qt, j)
                nc.scalar.activation(
                    pt[:, i, 0:ea], sp[:, 0:ea], AF.Exp, bias=negc[:]
                )
                nc.vector.tensor_scalar(
                    pt[:, i, ea:QW].bitcast(u8), sp[:, ea:QW],
                    A8, B8, ALU.mult, ALU.add,
                )
                if j % 2 == 1:
                    pend_d.append((pt, jp == 0))

                # --- aux: vtp fp8 copies (block 0), prev-block chain
                if qt > 0:
                    chain_ops(qt, j)

            # ---- block epilogue ----
            emit_O(NJP - 2, pts[NJP - 2])
            emit_O(NJP - 1, pts[NJP - 1])
            flush_d(99, minkeep=0)
            chain["d_prev"] = cur_d[0]
            chain["o_prev"] = get_o()
            chain["usb_prev"] = usb
            if qt == NQT - 1:
                # final tail: run the qt=3 chain inline
                for j in (0, 1, 2, 3, 4, 5, 6, 7, 8, 9, 10, 11):
                    chain_ops(qt + 1, j)

        lp.__exit__(None, None, None)

    nc.compile()
    return nc


def _get_nc():
    if "nc" not in _CACHE:
        _CACHE["nc"] = _build()
    return _CACHE["nc"]


def _prep_inputs(x, gamma, beta, w_qkv, b_qkv, w_proj, b_proj):
    x = np.ascontiguousarray(x, dtype=np.float32)
    w_qkv = np.asarray(w_qkv, dtype=np.float32)
    b_qkv = np.asarray(b_qkv, dtype=np.float32)
    w_proj = np.asarray(w_proj, dtype=np.float32)
    b_proj = np.asarray(b_proj, dtype=np.float32)

    wq = w_qkv[0:C, :]
    wk = w_qkv[C:2 * C, :]
    wv = w_qkv[2 * C:3 * C, :]
    bqv = b_qkv[0:C]
    bvv = b_qkv[2 * C:3 * C]

    wqT = np.ascontiguousarray((wq * SCALE).T)
    wkT = np.ascontiguousarray(wk.T)
    wuT = np.ascontiguousarray((w_proj @ wv).T)
    beff = (b_proj + w_proj @ bvv).astype(np.float32)

    ig = np.zeros((C, GROUPS), np.float32)
    ig[np.arange(C), np.arange(C) // GSIZE] = 1.0
    igt = np.ascontiguousarray(ig.T)
    ig = ig * (1.0 / GSIZE)

    common = {
        "gamma": np.asarray(gamma, np.float32).reshape(C, 1),
        "beta": np.asarray(beta, np.float32).reshape(C, 1),
        "bq": (bqv * SCALE).reshape(C, 1),
        "beff": beff.reshape(C, 1),
        "wqT": wqT,
        "wkT": wkT,
        "wuT": wuT,
        "ig": ig,
        "igt": igt,
    }
    in_maps = []
    for b in range(B):
        m = dict(common)
        m["x"] = np.ascontiguousarray(x[b].reshape(C, HW))
        in_maps.append(m)
    return in_maps


def kernel(x, gamma, beta, w_qkv, b_qkv, w_proj, b_proj):
    from concourse.bass_utils import run_bass_kernel_spmd

    nc = _get_nc()
    in_maps = _prep_inputs(x, gamma, beta, w_qkv, b_qkv, w_proj, b_proj)
    res = run_bass_kernel_spmd(nc, in_maps, list(range(B)))
    out = np.stack([res.results[b]["out"] for b in range(B)], axis=0)
    return out.reshape(B, C, H, W).astype(np.float32)
